# revision 16
# baseline (speedup 1.0000x reference)
"""Trainium2 Bass kernel for nn_CTransformerBlock_36876589203656 (point transformer).

8 NeuronCores: core c -> (batch b = c//2, query half h = c%2); params replicated.
Host Morton-sorts each batch's points for spatial locality. Device: mlinear
mixing matrices from the feature mean, KNN top-24 over a 384-wide sorted
window (PE negated-distance matmul + DVE max8/max_index/match_replace),
indirect-DMA gather of 64-float [feat|xyz|1] neighbor rows, and the two
two-layer MLPs + L1-normalized vector attention assembled via accumulating
PSUM matmuls in j-major layout. Host finishes: exact reference-order reorder
of each query's 24 neighbors, final f2-mlinear + residual, inverse perm.
"""
import os

import numpy as np

import concourse.bacc as bacc
import concourse.bass as bass
import concourse.mybir as mybir
from concourse import tile
from concourse.bass_utils import run_bass_kernel_spmd

f32 = mybir.dt.float32
u16 = mybir.dt.uint16
u32 = mybir.dt.uint32
AF = mybir.ActivationFunctionType
OP = mybir.AluOpType

B, N, DP, C, K = 4, 4096, 32, 128, 24
NT = 16             # query tiles per core
ROWS = 128 * K      # 3072 cols per tile, j-major: col = 128*j + q
W = 4096            # selection window = full N
PAY = 64            # payload rows [feat32|xyz3|one|pad]
ONE_ROW = DP        # payload row holding 1.0 (base-partition-legal)
XYZ0 = DP + 1       # xyz rows 33:36
EPS = 1e-5

_prog_cache = {}


def _morton(p, bits=6):
    g = np.clip((p * (1 << bits)).astype(np.int64), 0, (1 << bits) - 1)
    code = np.zeros(len(p), dtype=np.int64)
    for bb in range(bits):
        for d in range(3):
            code |= ((g[:, d] >> bb) & 1) << (3 * bb + d)
    return code


def _build_program():
    nc = bacc.Bacc("TRN2", target_bir_lowering=False, debug=False,
                   dynamic_dma_scratch_size=32768)

    def din(name, shape, dtype=f32):
        return nc.dram_tensor(name, shape, dtype, kind="ExternalInput")

    def dout(name, shape, dtype=f32):
        return nc.dram_tensor(name, shape, dtype, kind="ExternalOutput")

    tbl = din("tbl", [N, PAY])
    fxT = din("fxT", [PAY, N])
    fxTq = din("fxTq", [PAY, NT * 128])
    winq = din("winq", [4, N])               # xyz rows [0,x,y,z] full
    a0c = din("a0c", [128, NT], f32)         # per-tile window base (per core)
    fc1_wT = din("fc1_wT", [DP, C])
    fc1_b_row = din("fc1_b_row", [1, C])
    fc1_w = din("fc1_w", [C, DP])
    nfc1_w = din("nfc1_w", [C, DP])
    fc1_bc = din("fc1_bc", [C, 1])
    nfc1_bc = din("nfc1_bc", [C, 1])
    g1T = din("g1T", [C, C])
    g2T = din("g2T", [C, C])
    d2T = din("d2T", [C, C])
    D2GT = din("D2GT", [C, C])
    d1b = din("d1b", [4, C])
    Mh1 = din("Mh1", [PAY, C])
    b2d_row = din("b2d_row", [1, C])
    hb_row = din("hb_row", [1, C])
    b2g_row = din("b2g_row", [1, C])
    ones_1r = din("ones_1r", [1, 128])
    ones_row = din("ones_row", [1, 512])
    ones_col = din("ones_col", [128, 1])
    one1 = din("one1", [1, 1])
    I128 = din("I128", [128, 128])
    q2s = din("q2s", [4, 1])
    I4 = din("I4", [128, 512])
    mlw = {m: dict(W1T=din(f"{m}_W1T", [C, C]), W2T=din(f"{m}_W2T", [C, C]),
                   WcT=din(f"{m}_WcT", [C, C]), bc_row=din(f"{m}_bc_row", [1, C]))
           for m in ("q", "k", "v")}

    attn_o = dout("attn_o", [128, NT * ROWS])
    res_o = dout("res_o", [128, NT * 128])
    mi_o = dout("mi_o", [NT * 128, K], u16)

    with tile.TileContext(nc) as tc:
        with tc.tile_pool(name="const", bufs=1) as cp, \
             tc.tile_pool(name="persist", bufs=1) as pp, \
             tc.tile_pool(name="setup", bufs=2) as sp, \
             tc.tile_pool(name="work", bufs=2) as wp, \
             tc.tile_pool(name="psK", bufs=1, space="PSUM") as psK:

            def lc(t):
                tl = cp.tile(list(t.shape), t.dtype, name=t.name + "_c")
                nc.sync.dma_start(tl, t.ap())
                return tl

            FXT = pp.tile([PAY, N], f32, name="FXT")
            nc.sync.dma_start(FXT, fxT.ap())
            FXTQ = pp.tile([PAY, NT * 128], f32, name="FXTQ")
            nc.sync.dma_start(FXTQ, fxTq.ap())
            WINQ = pp.tile([4, N], f32, name="WINQ")
            nc.sync.dma_start(WINQ, winq.ap())
            A0C = pp.tile([128, NT], f32, name="A0C")
            nc.sync.dma_start(A0C, a0c.ap())

            c_fc1wT = lc(fc1_wT); c_fc1br = lc(fc1_b_row)
            c_fc1w = lc(fc1_w); c_nfc1w = lc(nfc1_w)
            c_fc1b = lc(fc1_bc); c_nfc1b = lc(nfc1_bc)
            c_g1T = lc(g1T); c_g2T = lc(g2T); c_d2T = lc(d2T); c_D2GT = lc(D2GT)
            c_d1b = lc(d1b); c_Mh1 = lc(Mh1)
            c_b2d = lc(b2d_row); c_hb = lc(hb_row); c_b2g = lc(b2g_row)
            c_1r = lc(ones_1r); c_or = lc(ones_row); c_oc = lc(ones_col)
            c_11 = lc(one1); c_I = lc(I128); c_q2s = lc(q2s); c_I4 = lc(I4)
            c_ml = {m: {k2: lc(v2) for k2, v2 in mlw[m].items()} for m in mlw}

            # ---- setup (scoped PSUM pool, freed before main loop) ----
            setup_ps = tc.alloc_tile_pool(name="psS", bufs=1, space="PSUM")
            psS = setup_ps
            WSQ = WINQ  # rows: [sq(filled below), x, y, z]
            sqx = sp.tile([4, N], f32, name="sqx")
            nc.vector.tensor_tensor(sqx, WINQ, WINQ, op=OP.mult)
            c_ones4 = cp.tile([4, 1], f32, name="c_ones4")
            nc.vector.memset(c_ones4, 1.0)
            for ch in range(N // 512):
                sq_ps = psS.tile([1, 512], f32, name="sq_ps", tag="s")
                nc.tensor.matmul(sq_ps, c_ones4, sqx[:, 512 * ch:512 * (ch + 1)],
                                 start=True, stop=True)
                nc.scalar.copy(WSQ[0:1, 512 * ch:512 * (ch + 1)], sq_ps)

            # ---- mean of x via affine-of-mean ----
            mfeat = sp.tile([PAY, 1], f32, name="mfeat")
            nc.vector.tensor_reduce(mfeat, FXT, mybir.AxisListType.X, OP.add)
            nc.vector.tensor_scalar_mul(mfeat, mfeat, 1.0 / N)
            mf_ps = psS.tile([128, 1], f32, name="mf_ps", tag="s")
            nc.tensor.matmul(mf_ps, c_fc1wT, mfeat[0:DP, :], start=True, stop=False)
            nc.tensor.matmul(mf_ps, c_fc1br, c_11, start=False, stop=True)
            MF = pp.tile([128, 1], f32, name="MF")
            nc.scalar.copy(MF, mf_ps)
            mfT_ps = psS.tile([1, 128], f32, name="mfT_ps", tag="s")
            nc.tensor.transpose(mfT_ps, MF, c_I)
            MFR = pp.tile([1, 128], f32, name="MFR")
            nc.scalar.copy(MFR, mfT_ps)

            # ---- mlinear wc3 (q/k/v) ----
            wc3 = {}
            for m in ("q", "k", "v"):
                w = c_ml[m]
                mfb_ps = psS.tile([128, 128], f32, name=f"mfb_{m}", tag="s")
                nc.tensor.matmul(mfb_ps, c_1r, MFR, start=True, stop=True)
                t1 = sp.tile([128, 128], f32, name=f"t1_{m}", tag="t1")
                nc.vector.tensor_scalar_mul(t1, w["W1T"], MF)
                t2 = sp.tile([128, 128], f32, name=f"t2_{m}", tag="t2")
                nc.vector.tensor_tensor(t2, w["W2T"], mfb_ps, op=OP.mult)
                wcT = sp.tile([128, 128], f32, name=f"wcT_{m}", tag="wcT")
                nc.vector.tensor_tensor(wcT, t1, t2, op=OP.subtract)
                wc2_ps = psS.tile([128, 128], f32, name=f"wc2_{m}", tag="s")
                nc.tensor.matmul(wc2_ps, c_1r, w["bc_row"], start=True, stop=False)
                nc.tensor.matmul(wc2_ps, wcT, w["WcT"], start=False, stop=True)
                absb = sp.tile([128, 128], f32, name=f"absb_{m}", tag="absb")
                dn = sp.tile([128, 1], f32, name=f"dn_{m}", tag="dn")
                nc.scalar.activation(absb, wc2_ps, AF.Abs, accum_out=dn)
                nc.vector.tensor_scalar_add(dn, dn, 128.0 * EPS)
                rcp = sp.tile([128, 1], f32, name=f"rcp_{m}", tag="rcp")
                nc.vector.reciprocal(rcp, dn)
                wc3m = pp.tile([128, 128], f32, name=f"wc3_{m}")
                nc.scalar.activation(wc3m, wc2_ps, AF.Copy, scale=rcp)
                wc3[m] = wc3m

            # ---- composed per-batch maps MK, MV, RQB ----
            def transpose_sb(src, nm):
                ps = psS.tile([128, 128], f32, name=nm + "_tps", tag="s")
                nc.tensor.transpose(ps, src, c_I)
                out = sp.tile([128, 128], f32, name=nm + "_T", tag="sbT")
                nc.scalar.copy(out, ps)
                return out

            wc3kT = transpose_sb(wc3["k"], "wc3k")
            wc3qT = transpose_sb(wc3["q"], "wc3q")
            sk_ps = psS.tile([128, 128], f32, name="sk_ps", tag="s")
            nc.tensor.matmul(sk_ps, wc3kT, c_g1T, start=True, stop=True)
            SK = sp.tile([128, 128], f32, name="SK")
            nc.scalar.copy(SK, sk_ps)
            rq_ps = psS.tile([128, 128], f32, name="rq_ps", tag="s")
            nc.tensor.matmul(rq_ps, wc3qT, c_g1T, start=True, stop=True)
            RQB = pp.tile([128, 128], f32, name="RQB")
            nc.scalar.copy(RQB, rq_ps)

            mk_ps = psS.tile([PAY, 128], f32, name="mk_ps", tag="s")
            nc.vector.memset(mk_ps, 0.0)
            nc.tensor.matmul(mk_ps[0:DP, :], c_nfc1w, SK, start=False, stop=False,
                             skip_group_check=True)
            nc.tensor.matmul(mk_ps[ONE_ROW:ONE_ROW + 1, :], c_nfc1b, SK,
                             start=False, stop=True, skip_group_check=True)
            MK = pp.tile([PAY, 128], f32, name="MK")
            nc.scalar.copy(MK, mk_ps)

            mv_ps = psS.tile([PAY, 128], f32, name="mv_ps", tag="s")
            nc.vector.memset(mv_ps, 0.0)
            nc.tensor.matmul(mv_ps[0:DP, :], c_fc1w, wc3["v"], start=False,
                             stop=False, skip_group_check=True)
            nc.tensor.matmul(mv_ps[ONE_ROW:ONE_ROW + 1, :], c_fc1b, wc3["v"],
                             start=False, stop=False, skip_group_check=True)
            nc.tensor.matmul(mv_ps[ONE_ROW:ONE_ROW + 1, :], c_11, c_b2d,
                             start=False, stop=True, skip_group_check=True)
            MV = pp.tile([PAY, 128], f32, name="MV")
            nc.scalar.copy(MV, mv_ps)
            setup_ps.release()
            psM = tc.alloc_tile_pool(name="psM", bufs=1, space="PSUM")

            # ---- main loop over query tiles ----
            for t in range(NT):
                qs = slice(128 * t, 128 * (t + 1))

                # T1 (h1pre query term) and QB (Z1 query term), row-major [q, oc]
                xyz1q = wp.tile([4, 128], f32, name="xyz1q", tag="xyz1q")
                nc.vector.tensor_copy(xyz1q, FXTQ[DP:DP + 4, qs])
                t1_ps = psM.tile([128, 128], f32, name="t1q_ps", tag="tq")
                nc.tensor.matmul(t1_ps, xyz1q, c_d1b,
                                 start=True, stop=True)
                T1Q = wp.tile([128, 128], f32, name="T1Q", tag="T1Q")
                nc.scalar.copy(T1Q, t1_ps)

                # QB needs x rows for these queries: x_q^T computed on the fly
                xq_ps = psM.tile([128, 128], f32, name="xq_ps", tag="tq")
                nc.tensor.matmul(xq_ps, c_fc1wT, FXTQ[0:DP, qs],
                                 start=True, stop=False)
                nc.tensor.matmul(xq_ps, c_fc1br, c_or[0:1, 0:128],
                                 start=False, stop=True)
                XQT = wp.tile([128, 128], f32, name="XQT", tag="XQT")
                nc.scalar.copy(XQT, xq_ps)
                qb_ps = psM.tile([128, 128], f32, name="qb_ps", tag="tq")
                nc.tensor.matmul(qb_ps, XQT, RQB, start=True, stop=False)
                nc.tensor.matmul(qb_ps, c_1r, c_hb, start=False, stop=True)
                QB = wp.tile([128, 128], f32, name="QB", tag="QB")
                nc.scalar.copy(QB, qb_ps)

                # ---- selection ----
                q2 = wp.tile([4, 128], f32, name="q2", tag="q2")
                nc.vector.tensor_scalar_mul(q2, FXTQ[DP:DP + 4, qs], c_q2s)
                ndb = wp.tile([128, W], f32, name="ndb", tag="ndb")
                for ch in range(8):
                    nd_ps = psK.tile([128, 512], f32, name="nd_ps", tag="nd")
                    nc.tensor.matmul(nd_ps, q2,
                                     WSQ[:, 512 * ch:512 * (ch + 1)],
                                     start=True, stop=True)
                    nc.scalar.copy(ndb[:, 512 * ch:512 * (ch + 1)], nd_ps)
                mval = wp.tile([128, 24], f32, name="mval", tag="mval")
                mi = wp.tile([128, 24], u16, name="mi", tag="mi")
                for r in range(3):
                    nc.vector.max(mval[:, 8 * r:8 * r + 8], ndb)
                    nc.vector.max_index(mi[:, 8 * r:8 * r + 8],
                                        mval[:, 8 * r:8 * r + 8], ndb)
                    if r < 2:
                        nc.vector.match_replace(ndb, mval[:, 8 * r:8 * r + 8],
                                                ndb, -3.0e38)
                nc.sync.dma_start(mi_o.ap()[qs, :], mi)
                offsf = wp.tile([128, 24], f32, name="offsf", tag="offsf")
                nc.vector.tensor_scalar(offsf, mi, A0C[:, t:t + 1], scalar2=None,
                                        op0=OP.add)
                offs = wp.tile([128, 24], u32, name="offs", tag="offs")
                nc.vector.tensor_copy(offs, offsf)

                # ---- per-bank MLP/attention (j-groups of 4) ----
                res_acc = wp.tile([128, 128], f32, name="res_acc", tag="res_acc")
                for i in range(6):
                    gtp = psK.tile([PAY, 512], f32, name="gtp", tag="gtp")
                    for jj in range(4):
                        j = 4 * i + jj
                        gt = wp.tile([128, PAY], f32, name=f"gt{jj}", tag=f"gt{jj}")
                        nc.gpsimd.indirect_dma_start(
                            out=gt, out_offset=None, in_=tbl.ap(),
                            in_offset=bass.IndirectOffsetOnAxis(
                                ap=offs[:, j:j + 1], axis=0))
                        nc.tensor.transpose(gtp[:, 128 * jj:128 * (jj + 1)],
                                            gt, c_I)
                    GTS = wp.tile([PAY, 512], f32, name="GTS", tag="GTS")
                    nc.scalar.copy(GTS, gtp)

                    h1_ps = psM.tile([128, 512], f32, name="h1_ps", tag="h1ps", bufs=2)
                    nc.tensor.matmul(h1_ps, T1Q, c_I4, start=True, stop=False,
                                     skip_group_check=True)
                    nc.tensor.matmul(h1_ps, c_Mh1, GTS, start=False, stop=True,
                                     skip_group_check=True)
                    H1 = wp.tile([128, 512], f32, name="H1", tag="H1")
                    nc.scalar.activation(H1, h1_ps, AF.Relu)

                    z1_ps = psM.tile([128, 512], f32, name="z1_ps", tag="z1ps")
                    nc.tensor.matmul(z1_ps, c_D2GT, H1, start=True, stop=False,
                                     skip_group_check=True)
                    nc.tensor.matmul(z1_ps, QB, c_I4, start=False, stop=False,
                                     skip_group_check=True)
                    nc.tensor.matmul(z1_ps, MK, GTS, start=False, stop=True,
                                     skip_group_check=True)
                    H2 = wp.tile([128, 512], f32, name="H2", tag="H2")
                    nc.scalar.activation(H2, z1_ps, AF.Relu)

                    au_ps = psM.tile([128, 512], f32, name="au_ps", tag="fin")
                    nc.tensor.matmul(au_ps, c_b2g, c_or, start=True, stop=False,
                                     skip_group_check=True)
                    nc.tensor.matmul(au_ps, c_g2T, H2, start=False, stop=True,
                                     skip_group_check=True)

                    vp_ps = psM.tile([128, 512], f32, name="vp_ps", tag="vpps")
                    nc.tensor.matmul(vp_ps, c_d2T, H1, start=True, stop=False,
                                     skip_group_check=True)
                    nc.tensor.matmul(vp_ps, MV, GTS, start=False, stop=True,
                                     skip_group_check=True)

                    ABSU = wp.tile([128, 512], f32, name="ABSU", tag="ABSU")
                    nc.scalar.activation(ABSU, au_ps, AF.Abs)
                    dn_ps = psM.tile([1, 512], f32, name="dn_ps", tag="fin")
                    nc.tensor.matmul(dn_ps, c_oc, ABSU, start=True, stop=True)
                    AUS = wp.tile([128, 512], f32, name="AUS", tag="AUS")
                    nc.scalar.copy(AUS, au_ps)

                    rc_i = wp.tile([1, 512], f32, name="rc_i", tag="rc_i")
                    nc.vector.tensor_scalar_add(rc_i, dn_ps, 128.0 * EPS)
                    nc.vector.reciprocal(rc_i, rc_i)
                    rb_ps = psM.tile([128, 512], f32, name="rb_ps", tag="fin")
                    nc.tensor.matmul(rb_ps, c_1r, rc_i, start=True, stop=True)
                    ATT = wp.tile([128, 512], f32, name="ATT", tag="ATT")
                    nc.vector.tensor_tensor(ATT, AUS, rb_ps, op=OP.mult)
                    nc.sync.dma_start(
                        attn_o.ap()[:, ROWS * t + 512 * i:ROWS * t + 512 * (i + 1)],
                        ATT)
                    TT_ = wp.tile([128, 512], f32, name="TT_", tag="TT_")
                    nc.vector.tensor_tensor(TT_, ATT, vp_ps, op=OP.mult)
                    part = wp.tile([128, 128], f32, name="part", tag="part")
                    tview = bass.AP(TT_.tensor, TT_.offset,
                                    [TT_.ap[0], [1, 128], [128, 4]])
                    nc.vector.tensor_reduce(part, tview, mybir.AxisListType.X,
                                            OP.add)
                    if i == 0:
                        nc.vector.tensor_copy(res_acc, part)
                    else:
                        nc.vector.tensor_tensor(res_acc, res_acc, part,
                                                op=OP.add)
                nc.sync.dma_start(res_o.ap()[:, qs], res_acc)
            psM.release()

    nc.compile()
    return nc


def _np_mlinear(feats, W1, W2, Wc, bc):
    mf = feats.mean(axis=0).astype(np.float32)
    wc = (mf[None, :] * W1 - mf[:, None] * W2).astype(np.float32)
    wc = (wc @ Wc.T + bc).astype(np.float32)
    wc = wc / (np.abs(wc) + EPS).sum(-1, keepdims=True)
    return (feats @ wc).astype(np.float32)


def kernel(**inputs):
    xyz = np.asarray(inputs["xyz"], dtype=np.float32)
    feats = np.asarray(inputs["features"], dtype=np.float32)
    P = {k: np.asarray(v, dtype=np.float32) for k, v in inputs.items()
         if k not in ("xyz", "features")}

    if "prog" not in _prog_cache:
        _prog_cache["prog"] = _build_program()
    nc = _prog_cache["prog"]

    # host-side sort
    perms, invs, stbl, sfxT = [], [], [], []
    for b in range(B):
        pm = np.argsort(_morton(xyz[b]), kind="stable")
        iv = np.empty(N, dtype=np.int64); iv[pm] = np.arange(N)
        perms.append(pm); invs.append(iv)
        t = np.zeros((N, PAY), np.float32)
        t[:, :DP] = feats[b][pm]
        t[:, ONE_ROW] = 1.0
        t[:, XYZ0:XYZ0 + 3] = xyz[b][pm]
        stbl.append(t); sfxT.append(np.ascontiguousarray(t.T))

    g1 = P["gamma_w1"]; g2 = P["gamma_w2"]
    d1 = P["delta_w1"]; d2 = P["delta_w2"]
    consts = dict(
        fc1_wT=np.ascontiguousarray(P["fc1_w"].T),
        fc1_b_row=P["fc1_b"][None, :].copy(),
        fc1_w=P["fc1_w"].copy(), nfc1_w=(-P["fc1_w"]).copy(),
        fc1_bc=P["fc1_b"][:, None].copy(), nfc1_bc=(-P["fc1_b"])[:, None].copy(),
        g1T=np.ascontiguousarray(g1.T), g2T=np.ascontiguousarray(g2.T),
        d2T=np.ascontiguousarray(d2.T),
        D2GT=np.ascontiguousarray((g1 @ d2).T),
        d1b=np.concatenate([P["delta_b1"][None, :], d1.T], 0).astype(np.float32),
        b2d_row=P["delta_b2"][None, :].copy(),
        hb_row=(P["gamma_b1"] + P["delta_b2"] @ g1.T)[None, :].astype(np.float32),
        b2g_row=P["gamma_b2"][None, :].copy(),
        ones_1r=np.ones((1, 128), np.float32),
        ones_row=np.ones((1, 512), np.float32),
        ones_col=np.ones((128, 1), np.float32),
        one1=np.ones((1, 1), np.float32),
        I128=np.eye(128, dtype=np.float32),
        q2s=np.array([[-1.0], [2.0], [2.0], [2.0]], np.float32),
        I4=np.tile(np.eye(128, dtype=np.float32), (1, 4)),
    )
    Mh1c = np.zeros((PAY, C), np.float32)
    Mh1c[XYZ0:XYZ0 + 3, :] = -d1.T
    consts["Mh1"] = Mh1c
    for m in ("q", "k", "v"):
        consts[f"{m}_W1T"] = np.ascontiguousarray(P[f"{m}_W1"].T)
        consts[f"{m}_W2T"] = np.ascontiguousarray(P[f"{m}_W2"].T)
        consts[f"{m}_WcT"] = np.ascontiguousarray(P[f"{m}_Wc"].T)
        consts[f"{m}_bc_row"] = P[f"{m}_bc"][None, :].copy()

    in_maps = []
    core_meta = []
    for c in range(8):
        b, h = c // 2, c % 2
        qoff0 = 2048 * h
        winq = np.zeros((4, N), np.float32)
        winq[1:4, :] = sfxT[b][XYZ0:XYZ0 + 3, :]
        a0s = np.zeros(NT, np.int64)
        fxtq = sfxT[b][:, qoff0:qoff0 + 2048].copy()
        m = dict(consts)
        m.update(tbl=stbl[b], fxT=sfxT[b], fxTq=fxtq,
                 winq=winq,
                 a0c=np.tile(a0s[None, :], (128, 1)).astype(np.float32))
        in_maps.append(m)
        core_meta.append((b, h, a0s))

    trace = bool(os.environ.get("KERNEL_TRACE"))
    res = run_bass_kernel_spmd(nc, in_maps, core_ids=list(range(8)),
                               trace=trace)
    if trace and res.exec_time_ns is not None:
        print(f"HW exec time: {res.exec_time_ns} ns")
    kernel._dbg = (res, core_meta, perms, stbl)

    # ---- host post-processing ----
    x_full = np.einsum("bnd,cd->bnc", feats, P["fc1_w"]) + P["fc1_b"]
    x_full = x_full.astype(np.float32)

    attn_full = np.zeros((B, N, K, C), np.float32)
    res_raw = np.zeros((B, N, C), np.float32)
    for c in range(8):
        b, h, a0s = core_meta[c]
        o = res.results[c]
        att = o["attn_o"]          # [128, NT*ROWS]
        rr = o["res_o"]            # [128, NT*128]
        mi = o["mi_o"].astype(np.int64)   # [NT*128, 24] window-relative
        pm = perms[b]
        sx = stbl[b][:, XYZ0:XYZ0 + 3]
        for t in range(NT):
            a0 = a0s[t]
            qsort = 2048 * h + 128 * t + np.arange(128)
            gidx = a0 + mi[128 * t:128 * (t + 1), :]          # sorted-space idx
            # exact reference ordering: d computed like reference, stable by orig idx
            qx = sx[qsort]                                    # [128, 3]
            nx = sx[gidx]                                     # [128, 24, 3]
            sqq = (qx * qx).sum(-1).astype(np.float32)
            sqn = np.einsum("qkd,qkd->qk", nx, nx).astype(np.float32)
            dot = np.einsum("qd,qkd->qk", qx, nx).astype(np.float32)
            dref = (sqq[:, None] + sqn - (2.0 * dot).astype(np.float32)
                    ).astype(np.float32)
            orig = pm[gidx]                                   # original indices
            order = np.lexsort((orig, dref), axis=-1)         # stable (d, idx)
            at = att[:, ROWS * t:ROWS * (t + 1)].reshape(128, K, 128)
            # at[c?, j, q] -> attn[q, j, c]
            at_q = at.transpose(2, 1, 0)                      # [q, j, c]
            qorig = pm[qsort]
            attn_full[b, qorig] = np.take_along_axis(
                at_q, order[:, :, None], axis=1)
            res_raw[b, qorig] = rr[:, 128 * t:128 * (t + 1)].T

    out_res = np.empty((B, N, C), np.float32)
    for b in range(B):
        out_res[b] = _np_mlinear(res_raw[b], P["f2_W1"], P["f2_W2"],
                                 P["f2_Wc"], P["f2_bc"]) + x_full[b]
    return out_res, attn_full


# revision 17
# speedup vs baseline: 1.0342x; 1.0342x over previous
"""Trainium2 Bass kernel for nn_CTransformerBlock_36876589203656 (point transformer).

8 NeuronCores: core c -> (batch b = c//2, query half h = c%2); params replicated.
Host Morton-sorts each batch's points for spatial locality. Device: mlinear
mixing matrices from the feature mean, KNN top-24 over a 384-wide sorted
window (PE negated-distance matmul + DVE max8/max_index/match_replace),
indirect-DMA gather of 64-float [feat|xyz|1] neighbor rows, and the two
two-layer MLPs + L1-normalized vector attention assembled via accumulating
PSUM matmuls in j-major layout. Host finishes: exact reference-order reorder
of each query's 24 neighbors, final f2-mlinear + residual, inverse perm.
"""
import os

import numpy as np

import concourse.bacc as bacc
import concourse.bass as bass
import concourse.mybir as mybir
from concourse import tile
from concourse.bass_utils import run_bass_kernel_spmd

f32 = mybir.dt.float32
u16 = mybir.dt.uint16
u32 = mybir.dt.uint32
AF = mybir.ActivationFunctionType
OP = mybir.AluOpType

B, N, DP, C, K = 4, 4096, 32, 128, 24
NT = 16             # query tiles per core
ROWS = 128 * K      # 3072 cols per tile, j-major: col = 128*j + q
W = 4096            # selection window = full N
PAY = 64            # payload rows [feat32|xyz3|one|pad]
ONE_ROW = DP        # payload row holding 1.0 (base-partition-legal)
XYZ0 = DP + 1       # xyz rows 33:36
EPS = 1e-5

_prog_cache = {}


def _morton(p, bits=6):
    g = np.clip((p * (1 << bits)).astype(np.int64), 0, (1 << bits) - 1)
    code = np.zeros(len(p), dtype=np.int64)
    for bb in range(bits):
        for d in range(3):
            code |= ((g[:, d] >> bb) & 1) << (3 * bb + d)
    return code


def _build_program():
    nc = bacc.Bacc("TRN2", target_bir_lowering=False, debug=False,
                   dynamic_dma_scratch_size=32768)

    def din(name, shape, dtype=f32):
        return nc.dram_tensor(name, shape, dtype, kind="ExternalInput")

    def dout(name, shape, dtype=f32):
        return nc.dram_tensor(name, shape, dtype, kind="ExternalOutput")

    tbl = din("tbl", [N, PAY])
    fxT = din("fxT", [PAY, N])
    fxTq = din("fxTq", [PAY, NT * 128])
    winq = din("winq", [4, N])               # xyz rows [0,x,y,z] full
    a0c = din("a0c", [128, NT], f32)         # per-tile window base (per core)
    fc1_wT = din("fc1_wT", [DP, C])
    fc1_b_row = din("fc1_b_row", [1, C])
    fc1_w = din("fc1_w", [C, DP])
    nfc1_w = din("nfc1_w", [C, DP])
    fc1_bc = din("fc1_bc", [C, 1])
    nfc1_bc = din("nfc1_bc", [C, 1])
    g1T = din("g1T", [C, C])
    g2T = din("g2T", [C, C])
    d2T = din("d2T", [C, C])
    D2GT = din("D2GT", [C, C])
    d1b = din("d1b", [4, C])
    Mh1 = din("Mh1", [PAY, C])
    b2d_row = din("b2d_row", [1, C])
    hb_row = din("hb_row", [1, C])
    b2g_row = din("b2g_row", [1, C])
    ones_1r = din("ones_1r", [1, 128])
    ones_row = din("ones_row", [1, 512])
    ones_col = din("ones_col", [128, 1])
    one1 = din("one1", [1, 1])
    I128 = din("I128", [128, 128])
    q2s = din("q2s", [4, 1])
    I4 = din("I4", [128, 512])
    mlw = {m: dict(W1T=din(f"{m}_W1T", [C, C]), W2T=din(f"{m}_W2T", [C, C]),
                   WcT=din(f"{m}_WcT", [C, C]), bc_row=din(f"{m}_bc_row", [1, C]))
           for m in ("q", "k", "v")}

    attn_o = dout("attn_o", [128, NT * ROWS])
    res_o = dout("res_o", [128, NT * 128])
    mi_o = dout("mi_o", [NT * 128, K], u16)

    with tile.TileContext(nc) as tc:
        with tc.tile_pool(name="const", bufs=1) as cp, \
             tc.tile_pool(name="persist", bufs=1) as pp, \
             tc.tile_pool(name="setup", bufs=2) as sp, \
             tc.tile_pool(name="work", bufs=2) as wp, \
             tc.tile_pool(name="psK", bufs=1, space="PSUM") as psK:

            def lc(t):
                tl = cp.tile(list(t.shape), t.dtype, name=t.name + "_c")
                nc.sync.dma_start(tl, t.ap())
                return tl

            FXT = pp.tile([PAY, N], f32, name="FXT")
            nc.sync.dma_start(FXT, fxT.ap())
            FXTQ = pp.tile([PAY, NT * 128], f32, name="FXTQ")
            nc.sync.dma_start(FXTQ, fxTq.ap())
            WINQ = pp.tile([4, N], f32, name="WINQ")
            nc.sync.dma_start(WINQ, winq.ap())
            A0C = pp.tile([128, NT], f32, name="A0C")
            nc.sync.dma_start(A0C, a0c.ap())

            c_fc1wT = lc(fc1_wT); c_fc1br = lc(fc1_b_row)
            c_fc1w = lc(fc1_w); c_nfc1w = lc(nfc1_w)
            c_fc1b = lc(fc1_bc); c_nfc1b = lc(nfc1_bc)
            c_g1T = lc(g1T); c_g2T = lc(g2T); c_d2T = lc(d2T); c_D2GT = lc(D2GT)
            c_d1b = lc(d1b); c_Mh1 = lc(Mh1)
            c_b2d = lc(b2d_row); c_hb = lc(hb_row); c_b2g = lc(b2g_row)
            c_1r = lc(ones_1r); c_or = lc(ones_row); c_oc = lc(ones_col)
            c_11 = lc(one1); c_I = lc(I128); c_q2s = lc(q2s); c_I4 = lc(I4)
            c_ml = {m: {k2: lc(v2) for k2, v2 in mlw[m].items()} for m in mlw}

            # ---- setup (scoped PSUM pool, freed before main loop) ----
            setup_ps = tc.alloc_tile_pool(name="psS", bufs=1, space="PSUM")
            psS = setup_ps
            WSQ = WINQ  # rows: [sq(filled below), x, y, z]
            sqx = sp.tile([4, N], f32, name="sqx")
            nc.vector.tensor_tensor(sqx, WINQ, WINQ, op=OP.mult)
            c_ones4 = cp.tile([4, 1], f32, name="c_ones4")
            nc.vector.memset(c_ones4, 1.0)
            for ch in range(N // 512):
                sq_ps = psS.tile([1, 512], f32, name="sq_ps", tag="s")
                nc.tensor.matmul(sq_ps, c_ones4, sqx[:, 512 * ch:512 * (ch + 1)],
                                 start=True, stop=True)
                nc.scalar.copy(WSQ[0:1, 512 * ch:512 * (ch + 1)], sq_ps)

            # ---- mean of x via affine-of-mean ----
            mfeat = sp.tile([PAY, 1], f32, name="mfeat")
            nc.vector.tensor_reduce(mfeat, FXT, mybir.AxisListType.X, OP.add)
            nc.vector.tensor_scalar_mul(mfeat, mfeat, 1.0 / N)
            mf_ps = psS.tile([128, 1], f32, name="mf_ps", tag="s")
            nc.tensor.matmul(mf_ps, c_fc1wT, mfeat[0:DP, :], start=True, stop=False)
            nc.tensor.matmul(mf_ps, c_fc1br, c_11, start=False, stop=True)
            MF = pp.tile([128, 1], f32, name="MF")
            nc.scalar.copy(MF, mf_ps)
            mfT_ps = psS.tile([1, 128], f32, name="mfT_ps", tag="s")
            nc.tensor.transpose(mfT_ps, MF, c_I)
            MFR = pp.tile([1, 128], f32, name="MFR")
            nc.scalar.copy(MFR, mfT_ps)

            # ---- mlinear wc3 (q/k/v) ----
            wc3 = {}
            for m in ("q", "k", "v"):
                w = c_ml[m]
                mfb_ps = psS.tile([128, 128], f32, name=f"mfb_{m}", tag="s")
                nc.tensor.matmul(mfb_ps, c_1r, MFR, start=True, stop=True)
                t1 = sp.tile([128, 128], f32, name=f"t1_{m}", tag="t1")
                nc.vector.tensor_scalar_mul(t1, w["W1T"], MF)
                t2 = sp.tile([128, 128], f32, name=f"t2_{m}", tag="t2")
                nc.vector.tensor_tensor(t2, w["W2T"], mfb_ps, op=OP.mult)
                wcT = sp.tile([128, 128], f32, name=f"wcT_{m}", tag="wcT")
                nc.vector.tensor_tensor(wcT, t1, t2, op=OP.subtract)
                wc2_ps = psS.tile([128, 128], f32, name=f"wc2_{m}", tag="s")
                nc.tensor.matmul(wc2_ps, c_1r, w["bc_row"], start=True, stop=False)
                nc.tensor.matmul(wc2_ps, wcT, w["WcT"], start=False, stop=True)
                absb = sp.tile([128, 128], f32, name=f"absb_{m}", tag="absb")
                dn = sp.tile([128, 1], f32, name=f"dn_{m}", tag="dn")
                nc.scalar.activation(absb, wc2_ps, AF.Abs, accum_out=dn)
                nc.vector.tensor_scalar_add(dn, dn, 128.0 * EPS)
                rcp = sp.tile([128, 1], f32, name=f"rcp_{m}", tag="rcp")
                nc.vector.reciprocal(rcp, dn)
                wc3m = pp.tile([128, 128], f32, name=f"wc3_{m}")
                nc.scalar.activation(wc3m, wc2_ps, AF.Copy, scale=rcp)
                wc3[m] = wc3m

            # ---- composed per-batch maps MK, MV, RQB ----
            def transpose_sb(src, nm):
                ps = psS.tile([128, 128], f32, name=nm + "_tps", tag="s")
                nc.tensor.transpose(ps, src, c_I)
                out = sp.tile([128, 128], f32, name=nm + "_T", tag="sbT")
                nc.scalar.copy(out, ps)
                return out

            wc3kT = transpose_sb(wc3["k"], "wc3k")
            wc3qT = transpose_sb(wc3["q"], "wc3q")
            sk_ps = psS.tile([128, 128], f32, name="sk_ps", tag="s")
            nc.tensor.matmul(sk_ps, wc3kT, c_g1T, start=True, stop=True)
            SK = sp.tile([128, 128], f32, name="SK")
            nc.scalar.copy(SK, sk_ps)
            rq_ps = psS.tile([128, 128], f32, name="rq_ps", tag="s")
            nc.tensor.matmul(rq_ps, wc3qT, c_g1T, start=True, stop=True)
            RQB = pp.tile([128, 128], f32, name="RQB")
            nc.scalar.copy(RQB, rq_ps)

            mk_ps = psS.tile([PAY, 128], f32, name="mk_ps", tag="s")
            nc.vector.memset(mk_ps, 0.0)
            nc.tensor.matmul(mk_ps[0:DP, :], c_nfc1w, SK, start=False, stop=False,
                             skip_group_check=True)
            nc.tensor.matmul(mk_ps[ONE_ROW:ONE_ROW + 1, :], c_nfc1b, SK,
                             start=False, stop=True, skip_group_check=True)
            MK = pp.tile([PAY, 128], f32, name="MK")
            nc.scalar.copy(MK, mk_ps)

            mv_ps = psS.tile([PAY, 128], f32, name="mv_ps", tag="s")
            nc.vector.memset(mv_ps, 0.0)
            nc.tensor.matmul(mv_ps[0:DP, :], c_fc1w, wc3["v"], start=False,
                             stop=False, skip_group_check=True)
            nc.tensor.matmul(mv_ps[ONE_ROW:ONE_ROW + 1, :], c_fc1b, wc3["v"],
                             start=False, stop=False, skip_group_check=True)
            nc.tensor.matmul(mv_ps[ONE_ROW:ONE_ROW + 1, :], c_11, c_b2d,
                             start=False, stop=True, skip_group_check=True)
            MV = pp.tile([PAY, 128], f32, name="MV")
            nc.scalar.copy(MV, mv_ps)
            setup_ps.release()
            psM = tc.alloc_tile_pool(name="psM", bufs=1, space="PSUM")

            # ---- main loop over query tiles ----
            for t in range(NT):
                qs = slice(128 * t, 128 * (t + 1))

                # T1 (h1pre query term) and QB (Z1 query term), row-major [q, oc]
                xyz1q = wp.tile([4, 128], f32, name="xyz1q", tag="xyz1q")
                nc.vector.tensor_copy(xyz1q, FXTQ[DP:DP + 4, qs])
                t1_ps = psM.tile([128, 128], f32, name="t1q_ps", tag="tq")
                nc.tensor.matmul(t1_ps, xyz1q, c_d1b,
                                 start=True, stop=True)
                T1Q = wp.tile([128, 128], f32, name="T1Q", tag="T1Q")
                nc.scalar.copy(T1Q, t1_ps)

                # QB needs x rows for these queries: x_q^T computed on the fly
                xq_ps = psM.tile([128, 128], f32, name="xq_ps", tag="tq")
                nc.tensor.matmul(xq_ps, c_fc1wT, FXTQ[0:DP, qs],
                                 start=True, stop=False)
                nc.tensor.matmul(xq_ps, c_fc1br, c_or[0:1, 0:128],
                                 start=False, stop=True)
                XQT = wp.tile([128, 128], f32, name="XQT", tag="XQT")
                nc.scalar.copy(XQT, xq_ps)
                qb_ps = psM.tile([128, 128], f32, name="qb_ps", tag="tq")
                nc.tensor.matmul(qb_ps, XQT, RQB, start=True, stop=False)
                nc.tensor.matmul(qb_ps, c_1r, c_hb, start=False, stop=True)
                QB = wp.tile([128, 128], f32, name="QB", tag="QB")
                nc.scalar.copy(QB, qb_ps)

                # ---- selection ----
                q2 = wp.tile([4, 128], f32, name="q2", tag="q2")
                nc.vector.tensor_scalar_mul(q2, FXTQ[DP:DP + 4, qs], c_q2s)
                ndb = wp.tile([128, W], f32, name="ndb", tag="ndb")
                for ch in range(8):
                    nd_ps = psK.tile([128, 512], f32, name="nd_ps", tag="nd")
                    nc.tensor.matmul(nd_ps, q2,
                                     WSQ[:, 512 * ch:512 * (ch + 1)],
                                     start=True, stop=True)
                    nc.scalar.copy(ndb[:, 512 * ch:512 * (ch + 1)], nd_ps)
                mval = wp.tile([128, 24], f32, name="mval", tag="mval")
                mi = wp.tile([128, 24], u16, name="mi", tag="mi")
                for r in range(3):
                    nc.vector.max(mval[:, 8 * r:8 * r + 8], ndb)
                    nc.vector.max_index(mi[:, 8 * r:8 * r + 8],
                                        mval[:, 8 * r:8 * r + 8], ndb)
                    if r < 2:
                        nc.vector.match_replace(ndb, mval[:, 8 * r:8 * r + 8],
                                                ndb, -3.0e38)
                nc.sync.dma_start(mi_o.ap()[qs, :], mi)
                offsf = wp.tile([128, 24], f32, name="offsf", tag="offsf")
                nc.vector.tensor_scalar(offsf, mi, A0C[:, t:t + 1], scalar2=None,
                                        op0=OP.add)
                offs = wp.tile([128, 24], u32, name="offs", tag="offs")
                nc.vector.tensor_copy(offs, offsf)

                # ---- per-bank MLP/attention (j-groups of 4) ----
                res_acc = wp.tile([128, 128], f32, name="res_acc", tag="res_acc")
                for i in range(6):
                    gtp = psK.tile([PAY, 512], f32, name="gtp", tag="gtp")
                    for jj in range(4):
                        j = 4 * i + jj
                        gt = wp.tile([128, PAY], f32, name=f"gt{jj}", tag=f"gt{jj}")
                        nc.gpsimd.indirect_dma_start(
                            out=gt, out_offset=None, in_=tbl.ap(),
                            in_offset=bass.IndirectOffsetOnAxis(
                                ap=offs[:, j:j + 1], axis=0))
                        nc.tensor.transpose(gtp[:, 128 * jj:128 * (jj + 1)],
                                            gt, c_I)
                    GTS = wp.tile([PAY, 512], f32, name="GTS", tag="GTS")
                    nc.scalar.copy(GTS, gtp)

                    h1_ps = psM.tile([128, 512], f32, name="h1_ps", tag="h1ps", bufs=2)
                    nc.tensor.matmul(h1_ps, T1Q, c_I4, start=True, stop=False,
                                     skip_group_check=True)
                    nc.tensor.matmul(h1_ps, c_Mh1, GTS, start=False, stop=True,
                                     skip_group_check=True)
                    H1 = wp.tile([128, 512], f32, name="H1", tag="H1")
                    nc.scalar.activation(H1, h1_ps, AF.Relu)

                    z1_ps = psM.tile([128, 512], f32, name="z1_ps", tag="z1ps")
                    nc.tensor.matmul(z1_ps, c_D2GT, H1, start=True, stop=False,
                                     skip_group_check=True)
                    nc.tensor.matmul(z1_ps, QB, c_I4, start=False, stop=False,
                                     skip_group_check=True)
                    nc.tensor.matmul(z1_ps, MK, GTS, start=False, stop=True,
                                     skip_group_check=True)
                    H2 = wp.tile([128, 512], f32, name="H2", tag="H2")
                    nc.vector.tensor_scalar_max(H2, z1_ps, 0.0)

                    au_ps = psM.tile([128, 512], f32, name="au_ps", tag="fin")
                    nc.tensor.matmul(au_ps, c_b2g, c_or, start=True, stop=False,
                                     skip_group_check=True)
                    nc.tensor.matmul(au_ps, c_g2T, H2, start=False, stop=True,
                                     skip_group_check=True)

                    vp_ps = psM.tile([128, 512], f32, name="vp_ps", tag="vpps")
                    nc.tensor.matmul(vp_ps, c_d2T, H1, start=True, stop=False,
                                     skip_group_check=True)
                    nc.tensor.matmul(vp_ps, MV, GTS, start=False, stop=True,
                                     skip_group_check=True)

                    ABSU = wp.tile([128, 512], f32, name="ABSU", tag="ABSU")
                    nc.scalar.activation(ABSU, au_ps, AF.Abs)
                    dn_ps = psM.tile([1, 512], f32, name="dn_ps", tag="fin")
                    nc.tensor.matmul(dn_ps, c_oc, ABSU, start=True, stop=True)
                    AUS = wp.tile([128, 512], f32, name="AUS", tag="AUS")
                    nc.scalar.copy(AUS, au_ps)

                    rc_i = wp.tile([1, 512], f32, name="rc_i", tag="rc_i")
                    nc.vector.tensor_scalar_add(rc_i, dn_ps, 128.0 * EPS)
                    nc.vector.reciprocal(rc_i, rc_i)
                    rb_ps = psM.tile([128, 512], f32, name="rb_ps", tag="fin")
                    nc.tensor.matmul(rb_ps, c_1r, rc_i, start=True, stop=True)
                    ATT = wp.tile([128, 512], f32, name="ATT", tag="ATT")
                    nc.vector.tensor_tensor(ATT, AUS, rb_ps, op=OP.mult)
                    nc.sync.dma_start(
                        attn_o.ap()[:, ROWS * t + 512 * i:ROWS * t + 512 * (i + 1)],
                        ATT)
                    TT_ = wp.tile([128, 512], f32, name="TT_", tag="TT_")
                    nc.vector.tensor_tensor(TT_, ATT, vp_ps, op=OP.mult)
                    part = wp.tile([128, 128], f32, name="part", tag="part")
                    tview = bass.AP(TT_.tensor, TT_.offset,
                                    [TT_.ap[0], [1, 128], [128, 4]])
                    nc.vector.tensor_reduce(part, tview, mybir.AxisListType.X,
                                            OP.add)
                    if i == 0:
                        nc.vector.tensor_copy(res_acc, part)
                    else:
                        nc.vector.tensor_tensor(res_acc, res_acc, part,
                                                op=OP.add)
                nc.sync.dma_start(res_o.ap()[:, qs], res_acc)
            psM.release()

    nc.compile()
    return nc


def _np_mlinear(feats, W1, W2, Wc, bc):
    mf = feats.mean(axis=0).astype(np.float32)
    wc = (mf[None, :] * W1 - mf[:, None] * W2).astype(np.float32)
    wc = (wc @ Wc.T + bc).astype(np.float32)
    wc = wc / (np.abs(wc) + EPS).sum(-1, keepdims=True)
    return (feats @ wc).astype(np.float32)


def kernel(**inputs):
    xyz = np.asarray(inputs["xyz"], dtype=np.float32)
    feats = np.asarray(inputs["features"], dtype=np.float32)
    P = {k: np.asarray(v, dtype=np.float32) for k, v in inputs.items()
         if k not in ("xyz", "features")}

    if "prog" not in _prog_cache:
        _prog_cache["prog"] = _build_program()
    nc = _prog_cache["prog"]

    # host-side sort
    perms, invs, stbl, sfxT = [], [], [], []
    for b in range(B):
        pm = np.argsort(_morton(xyz[b]), kind="stable")
        iv = np.empty(N, dtype=np.int64); iv[pm] = np.arange(N)
        perms.append(pm); invs.append(iv)
        t = np.zeros((N, PAY), np.float32)
        t[:, :DP] = feats[b][pm]
        t[:, ONE_ROW] = 1.0
        t[:, XYZ0:XYZ0 + 3] = xyz[b][pm]
        stbl.append(t); sfxT.append(np.ascontiguousarray(t.T))

    g1 = P["gamma_w1"]; g2 = P["gamma_w2"]
    d1 = P["delta_w1"]; d2 = P["delta_w2"]
    consts = dict(
        fc1_wT=np.ascontiguousarray(P["fc1_w"].T),
        fc1_b_row=P["fc1_b"][None, :].copy(),
        fc1_w=P["fc1_w"].copy(), nfc1_w=(-P["fc1_w"]).copy(),
        fc1_bc=P["fc1_b"][:, None].copy(), nfc1_bc=(-P["fc1_b"])[:, None].copy(),
        g1T=np.ascontiguousarray(g1.T), g2T=np.ascontiguousarray(g2.T),
        d2T=np.ascontiguousarray(d2.T),
        D2GT=np.ascontiguousarray((g1 @ d2).T),
        d1b=np.concatenate([P["delta_b1"][None, :], d1.T], 0).astype(np.float32),
        b2d_row=P["delta_b2"][None, :].copy(),
        hb_row=(P["gamma_b1"] + P["delta_b2"] @ g1.T)[None, :].astype(np.float32),
        b2g_row=P["gamma_b2"][None, :].copy(),
        ones_1r=np.ones((1, 128), np.float32),
        ones_row=np.ones((1, 512), np.float32),
        ones_col=np.ones((128, 1), np.float32),
        one1=np.ones((1, 1), np.float32),
        I128=np.eye(128, dtype=np.float32),
        q2s=np.array([[-1.0], [2.0], [2.0], [2.0]], np.float32),
        I4=np.tile(np.eye(128, dtype=np.float32), (1, 4)),
    )
    Mh1c = np.zeros((PAY, C), np.float32)
    Mh1c[XYZ0:XYZ0 + 3, :] = -d1.T
    consts["Mh1"] = Mh1c
    for m in ("q", "k", "v"):
        consts[f"{m}_W1T"] = np.ascontiguousarray(P[f"{m}_W1"].T)
        consts[f"{m}_W2T"] = np.ascontiguousarray(P[f"{m}_W2"].T)
        consts[f"{m}_WcT"] = np.ascontiguousarray(P[f"{m}_Wc"].T)
        consts[f"{m}_bc_row"] = P[f"{m}_bc"][None, :].copy()

    in_maps = []
    core_meta = []
    for c in range(8):
        b, h = c // 2, c % 2
        qoff0 = 2048 * h
        winq = np.zeros((4, N), np.float32)
        winq[1:4, :] = sfxT[b][XYZ0:XYZ0 + 3, :]
        a0s = np.zeros(NT, np.int64)
        fxtq = sfxT[b][:, qoff0:qoff0 + 2048].copy()
        m = dict(consts)
        m.update(tbl=stbl[b], fxT=sfxT[b], fxTq=fxtq,
                 winq=winq,
                 a0c=np.tile(a0s[None, :], (128, 1)).astype(np.float32))
        in_maps.append(m)
        core_meta.append((b, h, a0s))

    trace = bool(os.environ.get("KERNEL_TRACE"))
    res = run_bass_kernel_spmd(nc, in_maps, core_ids=list(range(8)),
                               trace=trace)
    if trace and res.exec_time_ns is not None:
        print(f"HW exec time: {res.exec_time_ns} ns")
    kernel._dbg = (res, core_meta, perms, stbl)

    # ---- host post-processing ----
    x_full = np.einsum("bnd,cd->bnc", feats, P["fc1_w"]) + P["fc1_b"]
    x_full = x_full.astype(np.float32)

    attn_full = np.zeros((B, N, K, C), np.float32)
    res_raw = np.zeros((B, N, C), np.float32)
    for c in range(8):
        b, h, a0s = core_meta[c]
        o = res.results[c]
        att = o["attn_o"]          # [128, NT*ROWS]
        rr = o["res_o"]            # [128, NT*128]
        mi = o["mi_o"].astype(np.int64)   # [NT*128, 24] window-relative
        pm = perms[b]
        sx = stbl[b][:, XYZ0:XYZ0 + 3]
        for t in range(NT):
            a0 = a0s[t]
            qsort = 2048 * h + 128 * t + np.arange(128)
            gidx = a0 + mi[128 * t:128 * (t + 1), :]          # sorted-space idx
            # exact reference ordering: d computed like reference, stable by orig idx
            qx = sx[qsort]                                    # [128, 3]
            nx = sx[gidx]                                     # [128, 24, 3]
            sqq = (qx * qx).sum(-1).astype(np.float32)
            sqn = np.einsum("qkd,qkd->qk", nx, nx).astype(np.float32)
            dot = np.einsum("qd,qkd->qk", qx, nx).astype(np.float32)
            dref = (sqq[:, None] + sqn - (2.0 * dot).astype(np.float32)
                    ).astype(np.float32)
            orig = pm[gidx]                                   # original indices
            order = np.lexsort((orig, dref), axis=-1)         # stable (d, idx)
            at = att[:, ROWS * t:ROWS * (t + 1)].reshape(128, K, 128)
            # at[c?, j, q] -> attn[q, j, c]
            at_q = at.transpose(2, 1, 0)                      # [q, j, c]
            qorig = pm[qsort]
            attn_full[b, qorig] = np.take_along_axis(
                at_q, order[:, :, None], axis=1)
            res_raw[b, qorig] = rr[:, 128 * t:128 * (t + 1)].T

    out_res = np.empty((B, N, C), np.float32)
    for b in range(B):
        out_res[b] = _np_mlinear(res_raw[b], P["f2_W1"], P["f2_W2"],
                                 P["f2_Wc"], P["f2_bc"]) + x_full[b]
    return out_res, attn_full


# revision 18
# speedup vs baseline: 1.1581x; 1.1197x over previous
"""Trainium2 Bass kernel for nn_CTransformerBlock_36876589203656 (point transformer).

8 NeuronCores: core c -> (batch b = c//2, query half h = c%2); params replicated.
Host Morton-sorts each batch's points for spatial locality. Device: mlinear
mixing matrices from the feature mean, KNN top-24 over a 384-wide sorted
window (PE negated-distance matmul + DVE max8/max_index/match_replace),
indirect-DMA gather of 64-float [feat|xyz|1] neighbor rows, and the two
two-layer MLPs + L1-normalized vector attention assembled via accumulating
PSUM matmuls in j-major layout. Host finishes: exact reference-order reorder
of each query's 24 neighbors, final f2-mlinear + residual, inverse perm.
"""
import os

import numpy as np

import concourse.bacc as bacc
import concourse.bass as bass
import concourse.mybir as mybir
from concourse import tile
from concourse.bass_utils import run_bass_kernel_spmd

f32 = mybir.dt.float32
bf16 = mybir.dt.bfloat16
u16 = mybir.dt.uint16
u32 = mybir.dt.uint32
AF = mybir.ActivationFunctionType
OP = mybir.AluOpType

B, N, DP, C, K = 4, 4096, 32, 128, 24
NT = 16             # query tiles per core
ROWS = 128 * K      # 3072 cols per tile, j-major: col = 128*j + q
W = 4096            # selection window = full N
PAY = 64            # payload rows [feat32|xyz3|one|pad]
ONE_ROW = DP        # payload row holding 1.0 (base-partition-legal)
XYZ0 = DP + 1       # xyz rows 33:36
EPS = 1e-5

_prog_cache = {}


def _morton(p, bits=6):
    g = np.clip((p * (1 << bits)).astype(np.int64), 0, (1 << bits) - 1)
    code = np.zeros(len(p), dtype=np.int64)
    for bb in range(bits):
        for d in range(3):
            code |= ((g[:, d] >> bb) & 1) << (3 * bb + d)
    return code


def _build_program():
    nc = bacc.Bacc("TRN2", target_bir_lowering=False, debug=False,
                   dynamic_dma_scratch_size=32768)

    def din(name, shape, dtype=f32):
        return nc.dram_tensor(name, shape, dtype, kind="ExternalInput")

    def dout(name, shape, dtype=f32):
        return nc.dram_tensor(name, shape, dtype, kind="ExternalOutput")

    tbl = din("tbl", [N, PAY])
    fxT = din("fxT", [PAY, N])
    fxTq = din("fxTq", [PAY, NT * 128])
    winq = din("winq", [4, N])               # xyz rows [0,x,y,z] full
    a0c = din("a0c", [128, NT], f32)         # per-tile window base (per core)
    fc1_wT = din("fc1_wT", [DP, C])
    fc1_b_row = din("fc1_b_row", [1, C])
    fc1_w = din("fc1_w", [C, DP])
    nfc1_w = din("nfc1_w", [C, DP])
    fc1_bc = din("fc1_bc", [C, 1])
    nfc1_bc = din("nfc1_bc", [C, 1])
    g1T = din("g1T", [C, C])
    g2T = din("g2T", [C, C])
    d2T = din("d2T", [C, C])
    D2GT = din("D2GT", [C, C])
    d1b = din("d1b", [4, C])
    Mh1 = din("Mh1", [PAY, C])
    b2d_row = din("b2d_row", [1, C])
    hb_row = din("hb_row", [1, C])
    b2g_row = din("b2g_row", [1, C])
    ones_1r = din("ones_1r", [1, 128])
    ones_row = din("ones_row", [1, 512])
    ones_col = din("ones_col", [128, 1])
    one1 = din("one1", [1, 1])
    I128 = din("I128", [128, 128])
    q2s = din("q2s", [4, 1])
    I4 = din("I4", [128, 512])
    mlw = {m: dict(W1T=din(f"{m}_W1T", [C, C]), W2T=din(f"{m}_W2T", [C, C]),
                   WcT=din(f"{m}_WcT", [C, C]), bc_row=din(f"{m}_bc_row", [1, C]))
           for m in ("q", "k", "v")}

    attn_o = dout("attn_o", [128, NT * ROWS])
    res_o = dout("res_o", [128, NT * 128])
    mi_o = dout("mi_o", [NT * 128, K], u16)

    with tile.TileContext(nc) as tc:
        with tc.tile_pool(name="const", bufs=1) as cp, \
             tc.tile_pool(name="persist", bufs=1) as pp, \
             tc.tile_pool(name="setup", bufs=2) as sp, \
             tc.tile_pool(name="work", bufs=2) as wp, \
             tc.tile_pool(name="psK", bufs=1, space="PSUM") as psK:

            def lc(t):
                tl = cp.tile(list(t.shape), t.dtype, name=t.name + "_c")
                nc.sync.dma_start(tl, t.ap())
                return tl

            FXT = pp.tile([PAY, N], f32, name="FXT")
            nc.sync.dma_start(FXT, fxT.ap())
            FXTQ = pp.tile([PAY, NT * 128], f32, name="FXTQ")
            nc.sync.dma_start(FXTQ, fxTq.ap())
            WINQ = pp.tile([4, N], f32, name="WINQ")
            nc.sync.dma_start(WINQ, winq.ap())
            A0C = pp.tile([128, NT], f32, name="A0C")
            nc.sync.dma_start(A0C, a0c.ap())

            c_fc1wT = lc(fc1_wT); c_fc1br = lc(fc1_b_row)
            c_fc1w = lc(fc1_w); c_nfc1w = lc(nfc1_w)
            c_fc1b = lc(fc1_bc); c_nfc1b = lc(nfc1_bc)
            c_g1T = lc(g1T); c_g2T = lc(g2T); c_d2T = lc(d2T); c_D2GT = lc(D2GT)
            c_d1b = lc(d1b); c_Mh1 = lc(Mh1)
            c_b2d = lc(b2d_row); c_hb = lc(hb_row); c_b2g = lc(b2g_row)
            c_1r = lc(ones_1r); c_or = lc(ones_row); c_oc = lc(ones_col)
            c_11 = lc(one1); c_I = lc(I128); c_q2s = lc(q2s); c_I4 = lc(I4)
            c_ml = {m: {k2: lc(v2) for k2, v2 in mlw[m].items()} for m in mlw}

            def bfc(src, nm):
                t = cp.tile(list(src.shape), bf16, name=nm)
                nc.vector.tensor_copy(t, src)
                return t

            b_D2GT = bfc(c_D2GT, "b_D2GT"); b_g2T = bfc(c_g2T, "b_g2T")
            b_d2T = bfc(c_d2T, "b_d2T"); b_Mh1 = bfc(c_Mh1, "b_Mh1")
            b_I4 = bfc(c_I4, "b_I4")

            # ---- setup (scoped PSUM pool, freed before main loop) ----
            setup_ps = tc.alloc_tile_pool(name="psS", bufs=1, space="PSUM")
            psS = setup_ps
            WSQ = WINQ  # rows: [sq(filled below), x, y, z]
            sqx = sp.tile([4, N], f32, name="sqx")
            nc.vector.tensor_tensor(sqx, WINQ, WINQ, op=OP.mult)
            c_ones4 = cp.tile([4, 1], f32, name="c_ones4")
            nc.vector.memset(c_ones4, 1.0)
            for ch in range(N // 512):
                sq_ps = psS.tile([1, 512], f32, name="sq_ps", tag="s")
                nc.tensor.matmul(sq_ps, c_ones4, sqx[:, 512 * ch:512 * (ch + 1)],
                                 start=True, stop=True)
                nc.scalar.copy(WSQ[0:1, 512 * ch:512 * (ch + 1)], sq_ps)

            # ---- mean of x via affine-of-mean ----
            mfeat = sp.tile([PAY, 1], f32, name="mfeat")
            nc.vector.tensor_reduce(mfeat, FXT, mybir.AxisListType.X, OP.add)
            nc.vector.tensor_scalar_mul(mfeat, mfeat, 1.0 / N)
            mf_ps = psS.tile([128, 1], f32, name="mf_ps", tag="s")
            nc.tensor.matmul(mf_ps, c_fc1wT, mfeat[0:DP, :], start=True, stop=False)
            nc.tensor.matmul(mf_ps, c_fc1br, c_11, start=False, stop=True)
            MF = pp.tile([128, 1], f32, name="MF")
            nc.scalar.copy(MF, mf_ps)
            mfT_ps = psS.tile([1, 128], f32, name="mfT_ps", tag="s")
            nc.tensor.transpose(mfT_ps, MF, c_I)
            MFR = pp.tile([1, 128], f32, name="MFR")
            nc.scalar.copy(MFR, mfT_ps)

            # ---- mlinear wc3 (q/k/v) ----
            wc3 = {}
            for m in ("q", "k", "v"):
                w = c_ml[m]
                mfb_ps = psS.tile([128, 128], f32, name=f"mfb_{m}", tag="s")
                nc.tensor.matmul(mfb_ps, c_1r, MFR, start=True, stop=True)
                t1 = sp.tile([128, 128], f32, name=f"t1_{m}", tag="t1")
                nc.vector.tensor_scalar_mul(t1, w["W1T"], MF)
                t2 = sp.tile([128, 128], f32, name=f"t2_{m}", tag="t2")
                nc.vector.tensor_tensor(t2, w["W2T"], mfb_ps, op=OP.mult)
                wcT = sp.tile([128, 128], f32, name=f"wcT_{m}", tag="wcT")
                nc.vector.tensor_tensor(wcT, t1, t2, op=OP.subtract)
                wc2_ps = psS.tile([128, 128], f32, name=f"wc2_{m}", tag="s")
                nc.tensor.matmul(wc2_ps, c_1r, w["bc_row"], start=True, stop=False)
                nc.tensor.matmul(wc2_ps, wcT, w["WcT"], start=False, stop=True)
                absb = sp.tile([128, 128], f32, name=f"absb_{m}", tag="absb")
                dn = sp.tile([128, 1], f32, name=f"dn_{m}", tag="dn")
                nc.scalar.activation(absb, wc2_ps, AF.Abs, accum_out=dn)
                nc.vector.tensor_scalar_add(dn, dn, 128.0 * EPS)
                rcp = sp.tile([128, 1], f32, name=f"rcp_{m}", tag="rcp")
                nc.vector.reciprocal(rcp, dn)
                wc3m = pp.tile([128, 128], f32, name=f"wc3_{m}")
                nc.scalar.activation(wc3m, wc2_ps, AF.Copy, scale=rcp)
                wc3[m] = wc3m

            # ---- composed per-batch maps MK, MV, RQB ----
            def transpose_sb(src, nm):
                ps = psS.tile([128, 128], f32, name=nm + "_tps", tag="s")
                nc.tensor.transpose(ps, src, c_I)
                out = sp.tile([128, 128], f32, name=nm + "_T", tag="sbT")
                nc.scalar.copy(out, ps)
                return out

            wc3kT = transpose_sb(wc3["k"], "wc3k")
            wc3qT = transpose_sb(wc3["q"], "wc3q")
            sk_ps = psS.tile([128, 128], f32, name="sk_ps", tag="s")
            nc.tensor.matmul(sk_ps, wc3kT, c_g1T, start=True, stop=True)
            SK = sp.tile([128, 128], f32, name="SK")
            nc.scalar.copy(SK, sk_ps)
            rq_ps = psS.tile([128, 128], f32, name="rq_ps", tag="s")
            nc.tensor.matmul(rq_ps, wc3qT, c_g1T, start=True, stop=True)
            RQB = pp.tile([128, 128], f32, name="RQB")
            nc.scalar.copy(RQB, rq_ps)

            mk_ps = psS.tile([PAY, 128], f32, name="mk_ps", tag="s")
            nc.vector.memset(mk_ps, 0.0)
            nc.tensor.matmul(mk_ps[0:DP, :], c_nfc1w, SK, start=False, stop=False,
                             skip_group_check=True)
            nc.tensor.matmul(mk_ps[ONE_ROW:ONE_ROW + 1, :], c_nfc1b, SK,
                             start=False, stop=True, skip_group_check=True)
            MK = pp.tile([PAY, 128], bf16, name="MK")
            nc.scalar.copy(MK, mk_ps)

            mv_ps = psS.tile([PAY, 128], f32, name="mv_ps", tag="s")
            nc.vector.memset(mv_ps, 0.0)
            nc.tensor.matmul(mv_ps[0:DP, :], c_fc1w, wc3["v"], start=False,
                             stop=False, skip_group_check=True)
            nc.tensor.matmul(mv_ps[ONE_ROW:ONE_ROW + 1, :], c_fc1b, wc3["v"],
                             start=False, stop=False, skip_group_check=True)
            nc.tensor.matmul(mv_ps[ONE_ROW:ONE_ROW + 1, :], c_11, c_b2d,
                             start=False, stop=True, skip_group_check=True)
            MV = pp.tile([PAY, 128], bf16, name="MV")
            nc.scalar.copy(MV, mv_ps)
            setup_ps.release()
            psM = tc.alloc_tile_pool(name="psM", bufs=1, space="PSUM")

            # ---- main loop over query tiles ----
            for t in range(NT):
                qs = slice(128 * t, 128 * (t + 1))

                # T1 (h1pre query term) and QB (Z1 query term), row-major [q, oc]
                xyz1q = wp.tile([4, 128], f32, name="xyz1q", tag="xyz1q")
                nc.vector.tensor_copy(xyz1q, FXTQ[DP:DP + 4, qs])
                t1_ps = psM.tile([128, 128], f32, name="t1q_ps", tag="tq")
                nc.tensor.matmul(t1_ps, xyz1q, c_d1b,
                                 start=True, stop=True)
                T1Q = wp.tile([128, 128], bf16, name="T1Q", tag="T1Q")
                nc.scalar.copy(T1Q, t1_ps)

                # QB needs x rows for these queries: x_q^T computed on the fly
                xq_ps = psM.tile([128, 128], f32, name="xq_ps", tag="tq")
                nc.tensor.matmul(xq_ps, c_fc1wT, FXTQ[0:DP, qs],
                                 start=True, stop=False)
                nc.tensor.matmul(xq_ps, c_fc1br, c_or[0:1, 0:128],
                                 start=False, stop=True)
                XQT = wp.tile([128, 128], f32, name="XQT", tag="XQT")
                nc.scalar.copy(XQT, xq_ps)
                qb_ps = psM.tile([128, 128], f32, name="qb_ps", tag="tq")
                nc.tensor.matmul(qb_ps, XQT, RQB, start=True, stop=False)
                nc.tensor.matmul(qb_ps, c_1r, c_hb, start=False, stop=True)
                QB = wp.tile([128, 128], bf16, name="QB", tag="QB")
                nc.scalar.copy(QB, qb_ps)

                # ---- selection ----
                q2 = wp.tile([4, 128], f32, name="q2", tag="q2")
                nc.vector.tensor_scalar_mul(q2, FXTQ[DP:DP + 4, qs], c_q2s)
                ndb = wp.tile([128, W], f32, name="ndb", tag="ndb")
                for ch in range(8):
                    nd_ps = psK.tile([128, 512], f32, name="nd_ps", tag="nd")
                    nc.tensor.matmul(nd_ps, q2,
                                     WSQ[:, 512 * ch:512 * (ch + 1)],
                                     start=True, stop=True)
                    nc.scalar.copy(ndb[:, 512 * ch:512 * (ch + 1)], nd_ps)
                mval = wp.tile([128, 24], f32, name="mval", tag="mval")
                mi = wp.tile([128, 24], u16, name="mi", tag="mi")
                for r in range(3):
                    nc.vector.max(mval[:, 8 * r:8 * r + 8], ndb)
                    nc.vector.max_index(mi[:, 8 * r:8 * r + 8],
                                        mval[:, 8 * r:8 * r + 8], ndb)
                    if r < 2:
                        nc.vector.match_replace(ndb, mval[:, 8 * r:8 * r + 8],
                                                ndb, -3.0e38)
                nc.sync.dma_start(mi_o.ap()[qs, :], mi)
                offsf = wp.tile([128, 24], f32, name="offsf", tag="offsf")
                nc.vector.tensor_scalar(offsf, mi, A0C[:, t:t + 1], scalar2=None,
                                        op0=OP.add)
                offs = wp.tile([128, 24], u32, name="offs", tag="offs")
                nc.vector.tensor_copy(offs, offsf)

                # ---- per-bank MLP/attention (j-groups of 4) ----
                res_acc = wp.tile([128, 128], f32, name="res_acc", tag="res_acc")
                for i in range(6):
                    gtp = psK.tile([PAY, 512], f32, name="gtp", tag="gtp")
                    for jj in range(4):
                        j = 4 * i + jj
                        gt = wp.tile([128, PAY], f32, name=f"gt{jj}", tag=f"gt{jj}")
                        nc.gpsimd.indirect_dma_start(
                            out=gt, out_offset=None, in_=tbl.ap(),
                            in_offset=bass.IndirectOffsetOnAxis(
                                ap=offs[:, j:j + 1], axis=0))
                        nc.tensor.transpose(gtp[:, 128 * jj:128 * (jj + 1)],
                                            gt, c_I)
                    GTS = wp.tile([PAY, 512], bf16, name="GTS", tag="GTS")
                    nc.scalar.copy(GTS, gtp)

                    h1_ps = psM.tile([128, 512], f32, name="h1_ps", tag="h1ps", bufs=2)
                    nc.tensor.matmul(h1_ps, T1Q, b_I4, start=True, stop=False,
                                     skip_group_check=True)
                    nc.tensor.matmul(h1_ps, b_Mh1, GTS, start=False, stop=True,
                                     skip_group_check=True)
                    H1 = wp.tile([128, 512], bf16, name="H1", tag="H1")
                    nc.scalar.activation(H1, h1_ps, AF.Relu)

                    z1_ps = psM.tile([128, 512], f32, name="z1_ps", tag="z1ps")
                    nc.tensor.matmul(z1_ps, b_D2GT, H1, start=True, stop=False,
                                     skip_group_check=True)
                    nc.tensor.matmul(z1_ps, QB, b_I4, start=False, stop=False,
                                     skip_group_check=True)
                    nc.tensor.matmul(z1_ps, MK, GTS, start=False, stop=True,
                                     skip_group_check=True)
                    H2 = wp.tile([128, 512], bf16, name="H2", tag="H2")
                    nc.vector.tensor_scalar_max(H2, z1_ps, 0.0)

                    au_ps = psM.tile([128, 512], f32, name="au_ps", tag="fin")
                    nc.tensor.matmul(au_ps, c_b2g, c_or, start=True, stop=False,
                                     skip_group_check=True)
                    nc.tensor.matmul(au_ps, b_g2T, H2, start=False, stop=True,
                                     skip_group_check=True)

                    vp_ps = psM.tile([128, 512], f32, name="vp_ps", tag="vpps")
                    nc.tensor.matmul(vp_ps, b_d2T, H1, start=True, stop=False,
                                     skip_group_check=True)
                    nc.tensor.matmul(vp_ps, MV, GTS, start=False, stop=True,
                                     skip_group_check=True)

                    ABSU = wp.tile([128, 512], f32, name="ABSU", tag="ABSU")
                    nc.scalar.activation(ABSU, au_ps, AF.Abs)
                    dn_ps = psM.tile([1, 512], f32, name="dn_ps", tag="fin")
                    nc.tensor.matmul(dn_ps, c_oc, ABSU, start=True, stop=True)
                    AUS = wp.tile([128, 512], f32, name="AUS", tag="AUS")
                    nc.scalar.copy(AUS, au_ps)

                    rc_i = wp.tile([1, 512], f32, name="rc_i", tag="rc_i")
                    nc.vector.tensor_scalar_add(rc_i, dn_ps, 128.0 * EPS)
                    nc.vector.reciprocal(rc_i, rc_i)
                    rb_ps = psM.tile([128, 512], f32, name="rb_ps", tag="fin")
                    nc.tensor.matmul(rb_ps, c_1r, rc_i, start=True, stop=True)
                    ATT = wp.tile([128, 512], f32, name="ATT", tag="ATT")
                    nc.vector.tensor_tensor(ATT, AUS, rb_ps, op=OP.mult)
                    nc.sync.dma_start(
                        attn_o.ap()[:, ROWS * t + 512 * i:ROWS * t + 512 * (i + 1)],
                        ATT)
                    TT_ = wp.tile([128, 512], f32, name="TT_", tag="TT_")
                    nc.vector.tensor_tensor(TT_, ATT, vp_ps, op=OP.mult)
                    part = wp.tile([128, 128], f32, name="part", tag="part")
                    tview = bass.AP(TT_.tensor, TT_.offset,
                                    [TT_.ap[0], [1, 128], [128, 4]])
                    nc.vector.tensor_reduce(part, tview, mybir.AxisListType.X,
                                            OP.add)
                    if i == 0:
                        nc.vector.tensor_copy(res_acc, part)
                    else:
                        nc.vector.tensor_tensor(res_acc, res_acc, part,
                                                op=OP.add)
                nc.sync.dma_start(res_o.ap()[:, qs], res_acc)
            psM.release()

    nc.compile()
    return nc


def _np_mlinear(feats, W1, W2, Wc, bc):
    mf = feats.mean(axis=0).astype(np.float32)
    wc = (mf[None, :] * W1 - mf[:, None] * W2).astype(np.float32)
    wc = (wc @ Wc.T + bc).astype(np.float32)
    wc = wc / (np.abs(wc) + EPS).sum(-1, keepdims=True)
    return (feats @ wc).astype(np.float32)


def kernel(**inputs):
    xyz = np.asarray(inputs["xyz"], dtype=np.float32)
    feats = np.asarray(inputs["features"], dtype=np.float32)
    P = {k: np.asarray(v, dtype=np.float32) for k, v in inputs.items()
         if k not in ("xyz", "features")}

    if "prog" not in _prog_cache:
        _prog_cache["prog"] = _build_program()
    nc = _prog_cache["prog"]

    # host-side sort
    perms, invs, stbl, sfxT = [], [], [], []
    for b in range(B):
        pm = np.argsort(_morton(xyz[b]), kind="stable")
        iv = np.empty(N, dtype=np.int64); iv[pm] = np.arange(N)
        perms.append(pm); invs.append(iv)
        t = np.zeros((N, PAY), np.float32)
        t[:, :DP] = feats[b][pm]
        t[:, ONE_ROW] = 1.0
        t[:, XYZ0:XYZ0 + 3] = xyz[b][pm]
        stbl.append(t); sfxT.append(np.ascontiguousarray(t.T))

    g1 = P["gamma_w1"]; g2 = P["gamma_w2"]
    d1 = P["delta_w1"]; d2 = P["delta_w2"]
    consts = dict(
        fc1_wT=np.ascontiguousarray(P["fc1_w"].T),
        fc1_b_row=P["fc1_b"][None, :].copy(),
        fc1_w=P["fc1_w"].copy(), nfc1_w=(-P["fc1_w"]).copy(),
        fc1_bc=P["fc1_b"][:, None].copy(), nfc1_bc=(-P["fc1_b"])[:, None].copy(),
        g1T=np.ascontiguousarray(g1.T), g2T=np.ascontiguousarray(g2.T),
        d2T=np.ascontiguousarray(d2.T),
        D2GT=np.ascontiguousarray((g1 @ d2).T),
        d1b=np.concatenate([P["delta_b1"][None, :], d1.T], 0).astype(np.float32),
        b2d_row=P["delta_b2"][None, :].copy(),
        hb_row=(P["gamma_b1"] + P["delta_b2"] @ g1.T)[None, :].astype(np.float32),
        b2g_row=P["gamma_b2"][None, :].copy(),
        ones_1r=np.ones((1, 128), np.float32),
        ones_row=np.ones((1, 512), np.float32),
        ones_col=np.ones((128, 1), np.float32),
        one1=np.ones((1, 1), np.float32),
        I128=np.eye(128, dtype=np.float32),
        q2s=np.array([[-1.0], [2.0], [2.0], [2.0]], np.float32),
        I4=np.tile(np.eye(128, dtype=np.float32), (1, 4)),
    )
    Mh1c = np.zeros((PAY, C), np.float32)
    Mh1c[XYZ0:XYZ0 + 3, :] = -d1.T
    consts["Mh1"] = Mh1c
    for m in ("q", "k", "v"):
        consts[f"{m}_W1T"] = np.ascontiguousarray(P[f"{m}_W1"].T)
        consts[f"{m}_W2T"] = np.ascontiguousarray(P[f"{m}_W2"].T)
        consts[f"{m}_WcT"] = np.ascontiguousarray(P[f"{m}_Wc"].T)
        consts[f"{m}_bc_row"] = P[f"{m}_bc"][None, :].copy()

    in_maps = []
    core_meta = []
    for c in range(8):
        b, h = c // 2, c % 2
        qoff0 = 2048 * h
        winq = np.zeros((4, N), np.float32)
        winq[1:4, :] = sfxT[b][XYZ0:XYZ0 + 3, :]
        a0s = np.zeros(NT, np.int64)
        fxtq = sfxT[b][:, qoff0:qoff0 + 2048].copy()
        m = dict(consts)
        m.update(tbl=stbl[b], fxT=sfxT[b], fxTq=fxtq,
                 winq=winq,
                 a0c=np.tile(a0s[None, :], (128, 1)).astype(np.float32))
        in_maps.append(m)
        core_meta.append((b, h, a0s))

    trace = bool(os.environ.get("KERNEL_TRACE"))
    res = run_bass_kernel_spmd(nc, in_maps, core_ids=list(range(8)),
                               trace=trace)
    if trace and res.exec_time_ns is not None:
        print(f"HW exec time: {res.exec_time_ns} ns")
    kernel._dbg = (res, core_meta, perms, stbl)

    # ---- host post-processing ----
    x_full = np.einsum("bnd,cd->bnc", feats, P["fc1_w"]) + P["fc1_b"]
    x_full = x_full.astype(np.float32)

    attn_full = np.zeros((B, N, K, C), np.float32)
    res_raw = np.zeros((B, N, C), np.float32)
    for c in range(8):
        b, h, a0s = core_meta[c]
        o = res.results[c]
        att = o["attn_o"]          # [128, NT*ROWS]
        rr = o["res_o"]            # [128, NT*128]
        mi = o["mi_o"].astype(np.int64)   # [NT*128, 24] window-relative
        pm = perms[b]
        sx = stbl[b][:, XYZ0:XYZ0 + 3]
        for t in range(NT):
            a0 = a0s[t]
            qsort = 2048 * h + 128 * t + np.arange(128)
            gidx = a0 + mi[128 * t:128 * (t + 1), :]          # sorted-space idx
            # exact reference ordering: d computed like reference, stable by orig idx
            qx = sx[qsort]                                    # [128, 3]
            nx = sx[gidx]                                     # [128, 24, 3]
            sqq = (qx * qx).sum(-1).astype(np.float32)
            sqn = np.einsum("qkd,qkd->qk", nx, nx).astype(np.float32)
            dot = np.einsum("qd,qkd->qk", qx, nx).astype(np.float32)
            dref = (sqq[:, None] + sqn - (2.0 * dot).astype(np.float32)
                    ).astype(np.float32)
            orig = pm[gidx]                                   # original indices
            order = np.lexsort((orig, dref), axis=-1)         # stable (d, idx)
            at = att[:, ROWS * t:ROWS * (t + 1)].reshape(128, K, 128)
            # at[c?, j, q] -> attn[q, j, c]
            at_q = at.transpose(2, 1, 0)                      # [q, j, c]
            qorig = pm[qsort]
            attn_full[b, qorig] = np.take_along_axis(
                at_q, order[:, :, None], axis=1)
            res_raw[b, qorig] = rr[:, 128 * t:128 * (t + 1)].T

    out_res = np.empty((B, N, C), np.float32)
    for b in range(B):
        out_res[b] = _np_mlinear(res_raw[b], P["f2_W1"], P["f2_W2"],
                                 P["f2_Wc"], P["f2_bc"]) + x_full[b]
    return out_res, attn_full
